# revision 1
# baseline (speedup 1.0000x reference)
"""Trainium2 Bass kernel for nn_AoAGNN (GATv2 GNN, 2 conv layers, attentional pooling).

Strategy (8 NeuronCores, SPMD):
  - Nodes partitioned into 8 contiguous ranges of 6250 (dst-range sharding).
  - Edges (incl. self loops) partitioned by dst core, sorted by dst, grouped
    into 128-dst-node blocks. Within each block, edges are split into
    src<32768 ("lo") and src>=32768 ("hi") groups, each padded to whole
    128-slot tiles, so per block the xl rows are fetched with TWO dma_gather
    ops (int16 indices into the lo/hi halves of the AllGathered xl table).
  - Per conv: node transforms per-shard; xl shard AllGathered in bf16;
    xr table kept resident in SBUF ([128, NBLK*C2], dst-block-major).
  - Edge phase per block: urg via one-hot PE matmuls + ACT evac, z = ulg+urg
    (DVE 2x), leaky-relu via fused scalar_tensor_tensor max(z, 0.2z), score =
    att-mult + segmented reduce, exp, mask; aggregation via one-hot PE
    matmuls with the softmax denominator folded in as 2 extra columns.
  - conv2 node transforms are fused into conv1's edge epilogue per block.
  - Pooling partials [64, 129] per core; host sums partials and runs the tiny
    head MLP + final normalize.
"""
import numpy as np
import ml_dtypes

import concourse.bass as bass
import concourse.mybir as mybir
import concourse.tile as tile
from concourse import bacc
from concourse.bass_utils import run_bass_kernel_spmd

F32 = mybir.dt.float32
BF16 = mybir.dt.bfloat16
I16 = mybir.dt.int16
AF = mybir.ActivationFunctionType
ALU = mybir.AluOpType
AX = mybir.AxisListType

N, E, IN, HID, HEADS, G = 50000, 500000, 256, 128, 2, 64
SLOPE = 0.2
NC = 8
BLK = 128
C2 = HEADS * HID         # 256
H16 = 32768              # int16 index limit -> lo/hi table split
NCN = N // NC            # 6250
NBLK = (NCN + BLK - 1) // BLK   # 49
LASTM = NCN - (NBLK - 1) * BLK  # 106
NPAD = NBLK * BLK

bf16 = ml_dtypes.bfloat16


def _bcast_mid(ap, t):
    """[128, F] AP -> [128, t, F] with step-0 middle dim."""
    (ps, pc), (fs, fc) = ap.ap
    return bass.AP(ap.tensor, ap.offset, [[ps, pc], [0, t], [fs, fc]])


def _bc(ap_col, n):
    (ps, pc), (fs, fc) = ap_col.ap
    return bass.AP(ap_col.tensor, ap_col.offset, [[ps, pc], [0, n]])


def _wrap16(a):
    """flat index list (len%16==0) -> [128, len/16] int16 (16-wrapped, 8x replicated)."""
    w = a.reshape(-1, 16).T.astype(np.int16)       # [16, n/16]
    return np.tile(w, (8, 1))


# ----------------------------------------------------------------------------
# Host-side preprocessing
# ----------------------------------------------------------------------------

def host_prep(inputs):
    x = np.asarray(inputs['x'], np.float32)
    ei = np.asarray(inputs['edge_index'], np.int64)
    batch = np.asarray(inputs['batch'], np.int64)

    src = np.concatenate([ei[0], np.arange(N, dtype=np.int64)])
    dst = np.concatenate([ei[1], np.arange(N, dtype=np.int64)])

    core_of = dst // NCN
    # per core, per block: lo/hi ordered edge lists
    per_core = []
    tlo = np.zeros((NC, NBLK), np.int64)
    thi = np.zeros((NC, NBLK), np.int64)
    for c in range(NC):
        m = core_of == c
        s_c = src[m]
        d_c = dst[m] - c * NCN
        order = np.argsort(d_c, kind='stable')
        s_c, d_c = s_c[order], d_c[order]
        b_c = d_c // BLK
        starts = np.searchsorted(b_c, np.arange(NBLK + 1))
        blocks = []
        for b in range(NBLK):
            lo, hi = starts[b], starts[b + 1]
            s_b = s_c[lo:hi]
            d_b = d_c[lo:hi] - b * BLK
            is_lo = s_b < H16
            s_lo, d_lo = s_b[is_lo], d_b[is_lo]
            s_hi, d_hi = s_b[~is_lo] - H16, d_b[~is_lo]
            blocks.append((s_lo, d_lo, s_hi, d_hi))
            tlo[c, b] = (len(s_lo) + BLK - 1) // BLK
            thi[c, b] = (len(s_hi) + BLK - 1) // BLK
        per_core.append(blocks)

    # shared (max-over-cores) tile structure
    TLO = tlo.max(axis=0)
    THI = thi.max(axis=0)
    TB = TLO + THI
    T = int(TB.max())
    CO = int(TB.sum())            # total tiles per conv
    CL = int(TLO.sum())
    CH = int(THI.sum())
    olo = np.concatenate([[0], np.cumsum(TLO)])   # per-block col offsets
    ohi = np.concatenate([[0], np.cumsum(THI)])
    ob = np.concatenate([[0], np.cumsum(TB)])

    ar = np.arange(BLK)
    eidxL_all, eidxH_all, oht_all, eqt_all, emsk_all, gseg_all, xT_all = \
        [], [], [], [], [], [], []
    for c in range(NC):
        eidxL = np.zeros((BLK, CL * 8), np.int16)
        eidxH = np.zeros((BLK, CH * 8), np.int16)
        oht = np.zeros((BLK, CO * BLK), bf16)
        eqt = np.zeros((BLK, CO * BLK), bf16)
        emsk = np.zeros((BLK, CO * 2), np.float32)
        for b in range(NBLK):
            s_lo, d_lo, s_hi, d_hi = per_core[c][b]
            nlo_s, nhi_s = TLO[b] * BLK, THI[b] * BLK
            ilo = np.zeros(nlo_s, np.int64); ilo[:len(s_lo)] = s_lo
            ihi = np.zeros(nhi_s, np.int64); ihi[:len(s_hi)] = s_hi
            segs = np.zeros(nlo_s + nhi_s, np.int64)
            segs[:len(d_lo)] = d_lo
            segs[nlo_s:nlo_s + len(d_hi)] = d_hi
            msk = np.zeros(nlo_s + nhi_s, np.float32)
            msk[:len(d_lo)] = 1.0
            msk[nlo_s:nlo_s + len(d_hi)] = 1.0
            if TLO[b]:
                eidxL[:, olo[b] * 8:olo[b + 1] * 8] = _wrap16(ilo)
            if THI[b]:
                eidxH[:, ohi[b] * 8:ohi[b + 1] * 8] = _wrap16(ihi)
            o = ob[b] * BLK
            nsl = nlo_s + nhi_s
            oht[:, o:o + nsl] = (segs[None, :] == ar[:, None])
            eq = (segs.reshape(TB[b], BLK)[:, :, None] == ar[None, None, :])
            eqt[:, o:o + nsl] = eq.transpose(1, 0, 2).reshape(BLK, nsl)
            emsk[:, ob[b] * 2:ob[b + 1] * 2] = \
                np.repeat(msk.reshape(TB[b], BLK).T, 2, axis=1)
        eidxL_all.append(eidxL)
        eidxH_all.append(eidxH)
        oht_all.append(oht)
        eqt_all.append(eqt)
        emsk_all.append(emsk)

        gseg = np.full((BLK, NBLK), 127.0, np.float32)
        bc_ = batch[c * NCN:(c + 1) * NCN].astype(np.float32)
        pad = np.full(NBLK * BLK - NCN, 127.0, np.float32)
        gseg[:, :] = np.concatenate([bc_, pad]).reshape(NBLK, BLK).T
        gseg_all.append(gseg.astype(bf16))

        xT_all.append(np.ascontiguousarray(x[c * NCN:(c + 1) * NCN].T).astype(bf16))

    struct = (T, CO, CL, CH, tuple(int(v) for v in TLO), tuple(int(v) for v in THI))
    return {'struct': struct,
            'eidxL': eidxL_all, 'eidxH': eidxH_all, 'oht': oht_all,
            'eqt': eqt_all, 'emsk': emsk_all, 'gseg': gseg_all, 'xT': xT_all}


def make_in_maps(inputs, host):
    inp = {k: np.asarray(v) for k, v in inputs.items()}
    shared = {}
    shared['giotaB'] = np.tile(np.arange(G, dtype=np.float32)[None, :], (BLK, 1)).astype(bf16)
    shared['identB'] = np.eye(BLK, dtype=np.float32).astype(bf16)
    shared['tinyC'] = np.full((BLK, 2), 1e-30, np.float32)
    shared['halfC'] = np.full((BLK, 2), 0.5, np.float32)
    shared['enc_w1T'] = np.ascontiguousarray(inp['enc_w1'].T).astype(bf16)   # [256,128]
    shared['enc_w2T'] = np.ascontiguousarray(inp['enc_w2'].T).astype(bf16)   # [128,128]
    shared['enc_b1c'] = inp['enc_b1'].astype(np.float32).reshape(HID, 1)
    shared['enc_b2c'] = inp['enc_b2'].astype(np.float32).reshape(HID, 1)
    for p in ('c1', 'c2'):
        for side in ('l', 'r'):
            w = inp[f'{p}_w{side}']          # [256, 128]
            shared[f'{p}_w{side}T'] = np.ascontiguousarray(w.T).astype(bf16)  # [128,256]
            b = inp[f'{p}_b{side}'].astype(np.float32)
            shared[f'{p}_b{side}B'] = np.tile(b[None, :], (BLK, 1))           # [128,256]
        shared[f'{p}_attB'] = np.tile(
            np.asarray(inp[f'{p}_att']).ravel()[None, :], (BLK, 1)).astype(bf16)
        shared[f'{p}_biasB'] = np.tile(inp[f'{p}_bias'].astype(np.float32)[None, :], (BLK, 1))
    shared['gate_w1T'] = np.ascontiguousarray(inp['gate_w1'].T).astype(bf16)  # [128,128]
    shared['gate_b1B'] = np.tile(inp['gate_b1'].astype(np.float32)[None, :], (BLK, 1))
    shared['gate_w2B'] = np.tile(inp['gate_w2'].astype(np.float32), (BLK, 1)).astype(bf16)

    in_maps = []
    import os as _os
    dum = None
    if int(_os.environ.get('KDUMTAB', '0')):
        dum = np.zeros((N, C2), bf16)
    for c in range(NC):
        m = dict(shared)
        for k in ('xT', 'eidxL', 'eidxH', 'oht', 'eqt', 'emsk', 'gseg'):
            m[k] = host[k][c]
        if dum is not None:
            m['dumtab'] = dum
        in_maps.append(m)
    return in_maps


# ----------------------------------------------------------------------------
# Device program
# ----------------------------------------------------------------------------

def build_program(struct):
    T, CO, CL, CH, TLO, THI = struct
    TB = [a + b for a, b in zip(TLO, THI)]
    olo = np.concatenate([[0], np.cumsum(TLO)]).astype(int)
    ohi = np.concatenate([[0], np.cumsum(THI)]).astype(int)
    ob = np.concatenate([[0], np.cumsum(TB)]).astype(int)

    nc = bacc.Bacc("TRN2", target_bir_lowering=False, debug=False,
                   enable_asserts=False, num_devices=NC)

    din = {}
    def ein(name, shape, dt):
        din[name] = nc.dram_tensor(name, list(shape), dt, kind="ExternalInput").ap()
        return din[name]

    ein('xT', (IN, NCN), BF16)
    ein('eidxL', (BLK, CL * 8), I16)
    ein('eidxH', (BLK, CH * 8), I16)
    ein('oht', (BLK, CO * BLK), BF16)
    ein('eqt', (BLK, CO * BLK), BF16)
    ein('emsk', (BLK, CO * 2), F32)
    ein('gseg', (BLK, NBLK), BF16)
    ein('giotaB', (BLK, G), BF16)
    ein('identB', (BLK, BLK), BF16)
    ein('tinyC', (BLK, 2), F32)
    ein('halfC', (BLK, 2), F32)
    ein('enc_w1T', (IN, HID), BF16)
    ein('enc_w2T', (HID, HID), BF16)
    ein('enc_b1c', (HID, 1), F32)
    ein('enc_b2c', (HID, 1), F32)
    for p in ('c1', 'c2'):
        ein(f'{p}_wlT', (HID, C2), BF16)
        ein(f'{p}_wrT', (HID, C2), BF16)
        ein(f'{p}_blB', (BLK, C2), F32)
        ein(f'{p}_brB', (BLK, C2), F32)
        ein(f'{p}_attB', (BLK, C2), BF16)
        ein(f'{p}_biasB', (BLK, HID), F32)
    ein('gate_w1T', (HID, HID), BF16)
    ein('gate_b1B', (BLK, HID), F32)
    ein('gate_w2B', (BLK, HID), BF16)

    pool_out = nc.dram_tensor("pool_out", [G, HID + 1], F32, kind="ExternalOutput").ap()
    import os as _os
    if int(_os.environ.get('KDUMTAB', '0')):
        ein('dumtab', (N, C2), BF16)

    RG = [list(range(NC))]
    SW = 258   # sulgb row width: HID*2 values + 2 denominator cols

    from contextlib import ExitStack
    with tile.TileContext(nc) as tc, ExitStack() as stk:
        cst = stk.enter_context(tc.tile_pool(name="cst", bufs=1))
        sb = {}
        for k in ('giotaB', 'identB', 'tinyC', 'halfC', 'enc_w2T',
                  'enc_b1c', 'enc_b2c', 'gate_w1T', 'gate_b1B', 'gate_w2B',
                  'c1_wlT', 'c1_wrT', 'c1_blB', 'c1_brB', 'c1_attB', 'c1_biasB',
                  'c2_wlT', 'c2_wrT', 'c2_blB', 'c2_brB', 'c2_attB', 'c2_biasB',
                  'eidxL', 'eidxH', 'emsk', 'gseg'):
            ap = din[k]
            t = cst.tile(list(ap.shape), ap.dtype, name=f"sb_{k}")
            nc.sync.dma_start(t[:], ap)
            sb[k] = t
        for half in range(2):
            t = cst.tile([BLK, HID], BF16, name=f"sb_enc_w1T{half}")
            nc.sync.dma_start(t[:], din['enc_w1T'][half * BLK:(half + 1) * BLK, :])
            sb[f'enc_w1T{half}'] = t

        # SBUF-resident xr tables (written by node phases, read by edge phases)
        tabRS = {}
        for p in ('c1', 'c2'):
            tabRS[p] = cst.tile([BLK, NBLK * C2], BF16, name=f"tabRS_{p}")
            # pad rows of the last (partial) node block must be finite; the
            # node phase overwrites rows [0, LASTM) afterwards
            nc.vector.memset(tabRS[p][:, (NBLK - 1) * C2:NBLK * C2], 0.0)

        hT_enc, _f1 = tc.tile([HID, NPAD], BF16, name="hT_enc")
        stk.callback(_f1)

        dram = stk.enter_context(tc.tile_pool(name="dram", bufs=1, space="DRAM"))
        tabs, shards = {}, {}
        for p in ('c1', 'c2'):
            tabs[p] = dram.tile([N, C2], BF16, name=f"tab_{p}", addr_space="Shared")
            shards[p] = dram.tile([NCN, C2], BF16, name=f"shard_{p}")

        # ------- encoder ---------------------------------------------------
        import os as _os2
        KSKIPPRE = _os2.environ.get('KSTAGE', 'E') == 'G'
        with tc.tile_pool(name="encp", bufs=2) as encp, \
             tc.tile_pool(name="encps", bufs=2, space="PSUM") as encps:
            xTs = []
            for half in range(2):
                xt = encp.tile([BLK, NCN], BF16, name=f"xT{half}", tag="xthalf")
                nc.sync.dma_start(xt[:], din['xT'][half * BLK:(half + 1) * BLK, :])
                xTs.append(xt)
            CHK = 512
            nch = 0 if KSKIPPRE else (NCN + CHK - 1) // CHK
            h1 = encp.tile([HID, NPAD], BF16, name="h1T", bufs=1)
            for i in range(nch):
                w = min(CHK, NCN - i * CHK)
                ps = encps.tile([HID, CHK], F32, tag="encps")
                for half in range(2):
                    nc.tensor.matmul(ps[:, :w],
                                     sb[f'enc_w1T{half}'][:],
                                     xTs[half][:, i * CHK:i * CHK + w],
                                     start=(half == 0), stop=(half == 1))
                nc.scalar.activation(h1[:, i * CHK:i * CHK + w], ps[:, :w],
                                     AF.Relu, bias=sb['enc_b1c'][:])
            if KSKIPPRE:
                nc.vector.memset(hT_enc[:], 0.0)
            for i in range(nch):
                w = min(CHK, NCN - i * CHK)
                ps = encps.tile([HID, CHK], F32, tag="encps")
                nc.tensor.matmul(ps[:, :w], sb['enc_w2T'][:],
                                 h1[:, i * CHK:i * CHK + w], start=True, stop=True)
                nc.scalar.activation(hT_enc[:, i * CHK:i * CHK + w], ps[:, :w],
                                     AF.Relu, bias=sb['enc_b2c'][:])

        # ------- conv1 node phase (from hT_enc) ----------------------------
        def node_block(p, lhs, M, nb, tb, tbps):
            """xl -> shard (DRAM, for AllGather); xr -> tabRS (SBUF)."""
            ps = tbps.tile([BLK, C2], F32, tag="tbps")
            nc.tensor.matmul(ps[:M], lhs, sb[f'{p}_wlT'][:], start=True, stop=True)
            tl = tb.tile([BLK, C2], BF16, tag="tbt")
            nc.vector.tensor_tensor(out=tl[:M], in0=ps[:M],
                                    in1=sb[f'{p}_blB'][:M], op=ALU.add)
            nc.sync.dma_start(shards[p][nb * BLK: nb * BLK + M, :], tl[:M])
            ps2 = tbps.tile([BLK, C2], F32, tag="tbps")
            nc.tensor.matmul(ps2[:M], lhs, sb[f'{p}_wrT'][:], start=True, stop=True)
            nc.vector.tensor_tensor(out=tabRS[p][:M, nb * C2:(nb + 1) * C2],
                                    in0=ps2[:M], in1=sb[f'{p}_brB'][:M], op=ALU.add)

        with tc.tile_pool(name="c1nb", bufs=3) as tb_, \
             tc.tile_pool(name="c1nps", bufs=2, space="PSUM") as tbps_:
            for nb in range(0 if KSKIPPRE else NBLK):
                M = BLK if nb < NBLK - 1 else LASTM
                node_block('c1', hT_enc[:, nb * BLK: nb * BLK + M], M, nb, tb_, tbps_)
        if not KSKIPPRE:
            nc.gpsimd.collective_compute(
                "AllGather", ALU.bypass, replica_groups=RG,
                ins=[shards['c1'][:].opt()], outs=[tabs['c1'][:].opt()])
        if int(__import__('os').environ.get('KBAR', '0')):
            nc.all_engine_barrier()

        # ------- edge phase ------------------------------------------------
        import os
        KEDGE = int(os.environ.get('KEDGE', '9'))

        def conv_edge(p, do_pool, fuse_next):
            tab = tabs[p]
            if int(__import__('os').environ.get('KDUMTAB', '0')):
                tab = din['dumtab']
            attB = sb[f'{p}_attB']
            with tc.tile_pool(name=f"{p}eg", bufs=3) as eg, \
                 tc.tile_pool(name=f"{p}es", bufs=2) as es, \
                 tc.tile_pool(name=f"{p}ea", bufs=2) as ea, \
                 tc.tile_pool(name=f"{p}eps", bufs=2, space="PSUM") as eps, \
                 tc.tile_pool(name=f"{p}aps", bufs=2, space="PSUM") as aps, \
                 tc.tile_pool(name=f"{p}tps", bufs=1, space="PSUM") as tps, \
                 tc.tile_pool(name=f"{p}fps", bufs=2, space="PSUM") as fps, \
                 tc.tile_pool(name=f"{p}gps", bufs=1, space="PSUM") as gps:
                if do_pool:
                    poolps = gps.tile([G, HID + 1], F32, name="poolps", bufs=1)
                for b in range(NBLK):
                    tb = TB[b]
                    tlo, thi = TLO[b], THI[b]
                    # --- gather xl rows (lo/hi halves, one dma_gather each)
                    ulg = eg.tile([BLK, T * C2], BF16, tag="ulg")
                    if KEDGE < 1:
                        nc.vector.memset(ulg[:], 0.0)
                    # dma_gather wedges the device above 1024 indices per
                    # call -> chunk to <=8 tiles per call
                    GMAX = 8
                    if KEDGE >= 1:
                        for half, tn, base, ix, tab_ap in (
                                (0, tlo, 0, 'eidxL', tab[0:H16, :]),
                                (1, thi, tlo, 'eidxH', tab[H16:N, :])):
                            off = (olo if half == 0 else ohi)[b]
                            for t0 in range(0, tn, GMAX):
                                tt = min(GMAX, tn - t0)
                                nc.gpsimd.dma_gather(
                                    out_ap=ulg[:, (base + t0) * C2:
                                               (base + t0 + tt) * C2].rearrange(
                                        "p (t c) -> p t c", c=C2),
                                    in_ap=tab_ap,
                                    idxs_ap=sb[ix][:, (off + t0) * 8:
                                                    (off + t0 + tt) * 8],
                                    num_idxs=tt * BLK, num_idxs_reg=tt * BLK,
                                    elem_size=C2)
                    if KEDGE == 1:
                        continue
                    # --- one-hot patterns
                    ohtb = ea.tile([BLK, T * BLK], BF16, tag="ohtb")
                    eqtb = ea.tile([BLK, T * BLK], BF16, tag="eqtb")
                    if KEDGE >= 2:
                        nc.sync.dma_start(
                            ohtb[:, :tb * BLK],
                            din['oht'][:, ob[b] * BLK:ob[b + 1] * BLK])
                        nc.sync.dma_start(
                            eqtb[:, :tb * BLK],
                            din['eqt'][:, ob[b] * BLK:ob[b + 1] * BLK])
                    # --- urg via one-hot matmuls, ACT evac
                    urgb = es.tile([BLK, T * C2], BF16, tag="urgb")
                    if KEDGE < 2:
                        nc.vector.memset(urgb[:], 0.0)
                    for t in range(tb if KEDGE >= 2 else 0):
                        ups = eps.tile([BLK, C2], F32, tag="ups")
                        nc.tensor.matmul(ups[:], ohtb[:, t * BLK:(t + 1) * BLK],
                                         tabRS[p][:, b * C2:(b + 1) * C2],
                                         start=True, stop=True)
                        nc.scalar.activation(urgb[:, t * C2:(t + 1) * C2],
                                             ups[:], AF.Copy)
                    # --- score path
                    SC = KEDGE >= 3
                    z = es.tile([BLK, T * C2], BF16, tag="z")
                    if not SC:
                        nc.vector.memset(z[:], 0.0)
                    if SC: nc.vector.tensor_tensor(out=z[:, :tb * C2],
                                            in0=ulg[:, :tb * C2],
                                            in1=urgb[:, :tb * C2], op=ALU.add)
                    lr = es.tile([BLK, T * C2], BF16, tag="lr")
                    if SC: nc.vector.scalar_tensor_tensor(
                        out=lr[:, :tb * C2], in0=z[:, :tb * C2], scalar=SLOPE,
                        in1=z[:, :tb * C2], op0=ALU.mult, op1=ALU.max)
                    lrat = es.tile([BLK, T * C2], BF16, tag="lrat")
                    if not SC: nc.vector.memset(lrat[:], 0.0)
                    if SC: nc.vector.tensor_tensor(out=lrat[:, :tb * C2],
                                            in0=lr[:, :tb * C2],
                                            in1=_bcast_mid(attB[:], tb),
                                            op=ALU.mult)
                    esc = es.tile([BLK, 2 * T], F32, tag="esc")
                    nc.vector.tensor_reduce(
                        out=esc[:, :2 * tb].rearrange("p (t h) -> p t h", h=HEADS),
                        in_=lrat[:, :tb * C2].rearrange(
                            "p (t h x) -> p t h x", h=HEADS, x=HID),
                        axis=AX.X, op=ALU.add)
                    wexp = es.tile([BLK, 2 * T], F32, tag="wexp")
                    nc.scalar.activation(wexp[:, :2 * tb], esc[:, :2 * tb], AF.Exp)
                    wm = es.tile([BLK, 2 * T], F32, tag="wm")
                    nc.vector.tensor_tensor(
                        out=wm[:, :2 * tb], in0=wexp[:, :2 * tb],
                        in1=sb['emsk'][:, ob[b] * 2:ob[b + 1] * 2], op=ALU.mult)
                    # --- weighted values + folded denominator
                    sulgb = es.tile([BLK, T * SW], BF16, tag="sulgb")
                    for t in range(tb if KEDGE >= 4 else 0):
                        for h in range(2):
                            nc.vector.tensor_scalar_mul(
                                out=sulgb[:, t * SW + h * HID:t * SW + (h + 1) * HID],
                                in0=ulg[:, t * C2 + h * HID:t * C2 + (h + 1) * HID],
                                scalar1=wm[:, 2 * t + h:2 * t + h + 1])
                    if KEDGE >= 4:
                        sap = sulgb[:]
                        den_ap = bass.AP(sap.tensor, sap.offset + 256,
                                         [list(sap.ap[0]), [SW, tb], [1, 2]])
                        nc.vector.tensor_copy(out=den_ap, in_=wm[:, :2 * tb])
                    # --- aggregation (PSUM accumulate across tiles)
                    aggp = aps.tile([BLK, SW], F32, tag="aggp")
                    for t in range(tb if KEDGE >= 4 else 0):
                        nc.tensor.matmul(aggp[:], eqtb[:, t * BLK:(t + 1) * BLK],
                                         sulgb[:, t * SW:(t + 1) * SW],
                                         start=(t == 0), stop=(t == tb - 1))
                    # --- epilogue: mean over heads / denom, bias, relu
                    denc = es.tile([BLK, 2], F32, tag="denc")
                    if KEDGE < 4:
                        nc.vector.memset(denc[:], 1.0)
                    if KEDGE >= 4: nc.vector.tensor_tensor(out=denc[:], in0=aggp[:, 256:258],
                                            in1=sb['tinyC'][:], op=ALU.max)
                    inv = es.tile([BLK, 2], F32, tag="inv")
                    nc.vector.reciprocal(inv[:], denc[:])
                    inv2 = es.tile([BLK, 2], F32, tag="inv2")
                    nc.vector.tensor_tensor(out=inv2[:], in0=inv[:],
                                            in1=sb['halfC'][:], op=ALU.mult)
                    t0 = es.tile([BLK, HID], F32, tag="t0")
                    yb = es.tile([BLK, HID + 1], F32, tag="yb")
                    if KEDGE < 4:
                        nc.vector.memset(yb[:], 0.0)
                    if KEDGE >= 4:
                        nc.vector.scalar_tensor_tensor(
                            out=t0[:], in0=aggp[:, 0:HID], scalar=inv2[:, 0:1],
                            in1=sb[f'{p}_biasB'][:], op0=ALU.mult, op1=ALU.add)
                        nc.vector.scalar_tensor_tensor(
                            out=yb[:, 0:HID], in0=aggp[:, HID:C2], scalar=inv2[:, 1:2],
                            in1=t0[:], op0=ALU.mult, op1=ALU.add)
                    hx = es.tile([BLK, HID + 1], BF16, tag="hx")
                    nc.scalar.activation(hx[:, 0:HID], yb[:, 0:HID], AF.Relu)
                    # transpose h for the (fused) next node phase / gate matmul
                    pt = tps.tile([BLK, BLK], BF16, tag="trps")
                    nc.tensor.transpose(pt[:], hx[:, 0:HID], sb['identB'][:])
                    hTb = es.tile([HID, BLK], BF16, tag="hTb")
                    nc.scalar.activation(hTb[:], pt[:], AF.Copy)
                    M = BLK if b < NBLK - 1 else LASTM
                    if fuse_next:
                        node_block(fuse_next, hTb[:, :M], M, b, es, fps)
                    if do_pool:
                        nc.vector.memset(hx[:, HID:HID + 1], 1.0)
                        g1ps = fps.tile([BLK, HID], F32, tag="g1ps")
                        nc.tensor.matmul(g1ps[:], hTb[:],
                                         sb['gate_w1T'][:], start=True, stop=True)
                        scr = es.tile([BLK, HID], F32, tag="scr")
                        nc.vector.tensor_tensor(out=scr[:], in0=g1ps[:],
                                                in1=sb['gate_b1B'][:], op=ALU.add)
                        g1rr = es.tile([BLK, HID], BF16, tag="g1rr")
                        nc.scalar.activation(g1rr[:], scr[:], AF.Relu)
                        scr2 = es.tile([BLK, HID], F32, tag="scr2")
                        nc.vector.tensor_tensor(out=scr2[:], in0=g1rr[:],
                                                in1=sb['gate_w2B'][:], op=ALU.mult)
                        gtb = es.tile([BLK, 1], F32, tag="gtb")
                        nc.vector.tensor_reduce(out=gtb[:], in_=scr2[:],
                                                axis=AX.X, op=ALU.add)
                        ge = es.tile([BLK, 1], F32, tag="ge")
                        nc.scalar.activation(ge[:], gtb[:], AF.Exp)
                        eqg = es.tile([BLK, G], BF16, tag="eqg")
                        nc.vector.tensor_tensor(out=eqg[:], in0=sb['giotaB'][:],
                                                in1=_bc(sb['gseg'][:, b:b + 1], G),
                                                op=ALU.is_equal)
                        Ag = es.tile([BLK, G], BF16, tag="Ag")
                        nc.vector.tensor_tensor(out=Ag[:], in0=eqg[:],
                                                in1=_bc(ge[:, 0:1], G), op=ALU.mult)
                        nc.tensor.matmul(poolps[:], Ag[:], hx[:],
                                         start=(b == 0), stop=(b == NBLK - 1))
                if do_pool:
                    po = es.tile([G, HID + 1], F32, name="po")
                    nc.vector.tensor_copy(out=po[:], in_=poolps[:])
                    nc.sync.dma_start(pool_out, po[:])

        import os
        stage = os.environ.get('KSTAGE', 'E')
        if stage in ('C', 'D', 'E', 'G'):
            conv_edge('c1', do_pool=False, fuse_next='c2' if stage != 'G' else None)
        if stage in ('D', 'E'):
            nc.gpsimd.collective_compute(
                "AllGather", ALU.bypass, replica_groups=RG,
                ins=[shards['c2'][:].opt()], outs=[tabs['c2'][:].opt()])
        if stage == 'E':
            conv_edge('c2', do_pool=True, fuse_next=None)
        else:
            with tc.tile_pool(name="dumo", bufs=1) as dp_:
                d_ = dp_.tile([G, HID + 1], F32, name="dummy_po")
                nc.vector.memset(d_[:], 1.0)
                nc.sync.dma_start(pool_out, d_[:])

    nc.compile()
    return nc


_CACHE = {}
LAST_RESULTS = None

def kernel(**inputs):
    host = host_prep(inputs)
    key = host['struct']
    if key not in _CACHE:
        _CACHE[key] = build_program(key)
    nc = _CACHE[key]
    in_maps = make_in_maps(inputs, host)
    import os
    trace = bool(int(os.environ.get('KTRACE', '0')))
    res = run_bass_kernel_spmd(nc, in_maps, core_ids=list(range(NC)),
                               trace=trace)
    global LAST_RESULTS
    LAST_RESULTS = res
    pool = np.zeros((G, HID + 1), np.float64)
    for r in res.results:
        pool += np.asarray(r['pool_out'], np.float64)
    g = (pool[:, :HID] / pool[:, HID:HID + 1]).astype(np.float32)
    out = (np.maximum(g @ np.asarray(inputs['head_w1'], np.float32).T
                      + np.asarray(inputs['head_b1'], np.float32), 0)
           @ np.asarray(inputs['head_w2'], np.float32).T
           + np.asarray(inputs['head_b2'], np.float32))
    out = out / np.maximum(np.linalg.norm(out, axis=1, keepdims=True), 1e-12)
    return out.astype(np.float32)

